# revision 22
# baseline (speedup 1.0000x reference)
"""Two-layer GAT (KeypointGraph) on 8 Trainium2 NeuronCores.

Strategy (dst-sharded message passing):
 - Host: add self-loops, partition edges by destination node into 8 cores x
   1088 dst nodes, split each core's dsts into 9 windows of 128; pack each
   window's edge list into 128-edge tiles; build per-tile one-hot matrices
   med/mde fed as one batched bf16 input per window.
 - Device (one NEFF, run once per GAT layer, SPMD on 8 cores):
   Phase H: every core computes the full augmented feature matmul
     H = X @ [W | W@a_src | W@a_dst] -> table rows [h(1024)|asrc(4)] bf16 in
     DRAM plus adst(4) f32 table.  h columns are stored HEAD-MINOR (col =
     4*c+h) so the per-tile softmax scaling is one packed-bf16 DVE multiply.
   Phase E: per 128-edge tile, indirect-DMA row gather of [h|asrc] by src id,
     adst via one-hot matmul, logits e = leaky_relu(asrc+adst) (fused
     scalar_tensor_tensor), ex = exp(e) -> bf16, msg = ex*h via ONE broadcast
     multiply [128,1024], then one-hot matmuls accumulate the per-window
     denominator [128,4] and output [128,1024] in PSUM across tiles.
     Window epilogue: yacc = sum_h po[:, h::4]*rec_h + bias via fused
     scalar_tensor_tensor chain -> Y f32.
 - Host between layers: x2 = relu(y1) -> rerun same NEFF with layer-2 weights.
"""

import sys

sys.path.insert(0, "/opt/trn_rl_repo")

import numpy as np
import ml_dtypes

import concourse.bass as bass
import concourse.mybir as mybir
import concourse.tile as tile
from concourse.ap import AP
from concourse.bass import ts
from concourse.bass_utils import run_bass_kernel_spmd

BF16 = ml_dtypes.bfloat16

B, K, F = 512, 17, 256
N = B * K              # 8704
HEADS, C = 4, 256
HC = HEADS * C         # 1024
NAUG = HC + 8          # 1032
NCORES = 8
NPC = N // NCORES      # 1088 dst nodes per core
NWIN = 9               # 8 full 128-dst windows + 1 half window
NPAD = 8832            # node table rows (8704 real + pad row 8704 + align)
PADROW = N             # gather index for padding edges

_cache = {}


def _split_multiwaits(nc):
    """This image's walrus supports only ONE sync-wait command per
    instruction; hoist extra waits onto prepended same-engine NoOps."""
    for f in nc.m.functions:
        for blk in f.blocks:
            old = blk.instructions
            new = []
            changed = False
            for inst in old:
                si = inst.sync_info
                if si is not None and len(si.on_wait) > 1:
                    waits = list(si.on_wait)
                    for k, w in enumerate(waits[:-1]):
                        new.append(
                            mybir.InstNoOp(
                                name=f"{inst.name}_wsplit{k}",
                                engine=inst.engine,
                                sync_info=mybir.SyncInfo(on_wait=[w], on_update=[]),
                                bass_nofuse=True,
                            )
                        )
                    inst.sync_info = mybir.SyncInfo(
                        on_wait=[waits[-1]], on_update=list(si.on_update)
                    )
                    changed = True
                new.append(inst)
            if changed:
                blk.instructions = new


def _bcast_ap(t, offset_cols, reps, inner):
    """AP over tile t: [partitions, (0,reps), (1,inner)] at column offset."""
    full = t[:, :]
    dims = [list(full.ap[0]), [0, reps], [1, inner]]
    return AP(full.tensor, full.offset + offset_cols, dims)


def _strided_ap(t, offset_cols, stride, count):
    full = t[:, :]
    dims = [list(full.ap[0]), [stride, count]]
    return AP(full.tensor, full.offset + offset_cols, dims)


def _build_layer_nc(tw):
    """One GAT layer, SPMD over 8 cores. tw: tiles per window (len NWIN)."""
    nc = bass.Bass(num_devices=NCORES)
    dt = mybir.dt
    Alu = mybir.AluOpType
    Act = mybir.ActivationFunctionType
    twmax = max(tw)

    XT = nc.dram_tensor("xt", [2, 128, NPAD], dt.bfloat16, kind="ExternalInput")
    WAUG = nc.dram_tensor("waug", [2, 128, NAUG], dt.bfloat16, kind="ExternalInput")
    BIAS = nc.dram_tensor("bias", [128, C], dt.float32, kind="ExternalInput")
    SRC = nc.dram_tensor("src", [NWIN, 128, twmax], dt.int32, kind="ExternalInput")
    ADIX = nc.dram_tensor("adix", [NWIN, 128, 1], dt.int32, kind="ExternalInput")
    MEDE = nc.dram_tensor("mede", [NWIN, 128, twmax * 256], dt.bfloat16, kind="ExternalInput")
    Y = nc.dram_tensor("y", [NWIN, 128, C], dt.float32, kind="ExternalOutput")

    TROWB = HC + 4  # bf16 elems: 1024 h | 4 asrc
    HTAB = nc.dram_tensor("htab", [NPAD, TROWB], dt.bfloat16)
    ADSTT = nc.dram_tensor("adstt", [NPAD, 4], dt.float32)

    with tile.TileContext(nc) as tc:
        with (
            tc.tile_pool(name="per", bufs=1) as per,
            tc.tile_pool(name="hsb", bufs=3) as hpool,
            tc.tile_pool(name="ed", bufs=8) as ed,
            tc.tile_pool(name="mp", bufs=2) as mp,
            tc.tile_pool(name="sm", bufs=8) as sm,
            tc.tile_pool(name="msp", bufs=6) as msp,
            tc.tile_pool(name="pph", bufs=2, space="PSUM") as pph,
            tc.tile_pool(name="pp1", bufs=2, space="PSUM") as pp1,
            tc.tile_pool(name="ppd", bufs=1, space="PSUM") as ppd,
            tc.tile_pool(name="pp2", bufs=1, space="PSUM") as pp2,
        ):
            xts = []
            for k in range(2):
                x = per.tile([128, NPAD], dt.bfloat16, tag=f"xt{k}")
                nc.sync.dma_start(x[:], XT[k])
                xts.append(x)
            wgs = []
            for k in range(2):
                w = per.tile([128, NAUG], dt.bfloat16, tag=f"wg{k}")
                nc.sync.dma_start(w[:], WAUG[k])
                wgs.append(w)
            bia = per.tile([128, C], dt.float32, tag="bias")
            nc.sync.dma_start(bia[:], BIAS[:])

            # ---- Phase H: augmented feature matmul into DRAM tables ----
            # GRP blocks staged per hsb tile, written with one (transposed-AP)
            # DMA each for htab and adstt.
            GRP = 4
            HT3 = HTAB[:, :].tensor
            for g0 in range(0, NPAD // 128, GRP):
                gn = min(GRP, NPAD // 128 - g0)
                hsb = hpool.tile([128, GRP, TROWB], dt.bfloat16, tag="hsb")
                asb = hpool.tile([128, GRP, 4], dt.float32, tag="asb")
                for b in range(gn):
                    nb = g0 + b
                    for c0, cn in ((0, 512), (512, 512), (1024, 8)):
                        ps = pph.tile([128, cn], dt.float32, tag="hps")
                        for k in range(2):
                            nc.tensor.matmul(
                                ps[:],
                                lhsT=xts[k][:, ts(nb, 128)],
                                rhs=wgs[k][:, c0 : c0 + cn],
                                start=(k == 0),
                                stop=(k == 1),
                            )
                        if cn == 512:
                            if (c0 == 0) == (nb % 2 == 0):
                                nc.scalar.copy(hsb[:, b, c0 : c0 + 512], ps[:])
                            else:
                                nc.vector.tensor_copy(hsb[:, b, c0 : c0 + 512], ps[:])
                        else:
                            nc.scalar.copy(hsb[:, b, 1024:1028], ps[:, 0:4])
                            nc.scalar.copy(asb[:, b, :], ps[:, 4:8])
                # htab rows g0*128 .. (g0+gn)*128: [part, block, col] AP
                ho = AP(
                    HTAB[:, :].tensor,
                    g0 * 128 * TROWB,
                    [[TROWB, 128], [128 * TROWB, gn], [1, TROWB]],
                )
                nc.sync.dma_start(ho, hsb[:, 0:gn, :])
                ao = AP(
                    ADSTT[:, :].tensor,
                    g0 * 128 * 4,
                    [[4, 128], [128 * 4, gn], [1, 4]],
                )
                nc.sync.dma_start(ao, asb[:, 0:gn, :])

            # ---- Phase E: per-window edge aggregation ----
            t0 = 0
            for w in range(NWIN):
                twn = tw[w]
                aidx = sm.tile([128, 1], dt.int32, tag="aidx")
                nc.sync.dma_start(aidx[:], ADIX[w])
                adw = sm.tile([128, 4], dt.float32, tag="adw")
                nc.gpsimd.indirect_dma_start(
                    out=adw[:],
                    out_offset=None,
                    in_=ADSTT[:, :],
                    in_offset=bass.IndirectOffsetOnAxis(ap=aidx[:, :1], axis=0),
                )
                adwb = sm.tile([128, 4], dt.bfloat16, tag="adwb")
                nc.vector.tensor_copy(adwb[:], adw[:])

                po0 = pp1.tile([128, 512], dt.float32, tag="po0")
                po1 = pp1.tile([128, 512], dt.float32, tag="po1")
                pos = (po0, po1)
                den = ppd.tile([128, 4], dt.float32, tag="den")

                sidxw = sm.tile([128, twmax], dt.int32, tag="sidxw")
                nc.sync.dma_start(sidxw[:], SRC[w])
                mt = mp.tile([128, twmax * 256], dt.bfloat16, tag="mt")
                nc.sync.dma_start(mt[:, : twn * 256], MEDE[w][:, : twn * 256])

                # gather all tiles' rows into one window tile (one single-index
                # indirect DMA per 128-edge tile, strided output slices)
                gw = mp.tile([128, twmax, TROWB], dt.bfloat16, tag="gw")
                for tl in range(twn):
                    nc.gpsimd.indirect_dma_start(
                        out=gw[:, tl, :],
                        out_offset=None,
                        in_=HTAB[:, :],
                        in_offset=bass.IndirectOffsetOnAxis(
                            ap=sidxw[:, tl : tl + 1], axis=0
                        ),
                    )

                # batched per-window logits: psa into col slices of one bank
                psa = pp2.tile([128, 4 * twmax], dt.float32, tag="psa")
                for tl in range(twn):
                    nc.tensor.matmul(
                        psa[:, 4 * tl : 4 * tl + 4],
                        lhsT=mt[:, tl * 256 + 128 : tl * 256 + 256],
                        rhs=adwb[:],
                        start=True,
                        stop=True,
                    )
                ef = sm.tile([128, twmax, 4], dt.float32, tag="ef")
                nc.vector.tensor_add(
                    ef[:, 0:twn],
                    gw[:, 0:twn, HC : HC + 4],
                    AP(psa.tensor, psa.offset, [list(psa.ap[0]), [4, twn], [1, 4]]),
                )
                nc.vector.scalar_tensor_tensor(
                    ef[:, 0:twn], ef[:, 0:twn], 0.2, ef[:, 0:twn], Alu.mult, Alu.max
                )
                exw = sm.tile([128, twmax, 4], dt.bfloat16, tag="exw")
                nc.scalar.activation(exw[:, 0:twn], ef[:, 0:twn], Act.Exp)

                for tl in range(twn):
                    first = tl == 0
                    last = tl == twn - 1
                    med = mt[:, tl * 256 : tl * 256 + 128]
                    # scale the ONE-HOT (not the messages): ms[,h,d] =
                    # med[,d] * ex[,h] in one fused mul (med repeated 4x via
                    # stride-0 outer dim), so the gathered rows feed matmuls
                    # unscaled (enables fp8 rhs).
                    ms = msp.tile([128, HEADS, 128], dt.bfloat16, tag="ms")
                    mtf = mt[:, :]
                    exf = exw[:, :]
                    in1 = AP(mtf.tensor, mtf.offset + tl * 256,
                             [list(mtf.ap[0]), [0, HEADS], [1, 128]])
                    in2 = AP(exf.tensor, exf.offset + 4 * tl,
                             [list(exf.ap[0]), [1, HEADS], [0, 128]])
                    nc.vector.tensor_mul(ms[:], in1, in2)
                    nc.tensor.matmul(
                        den[:], lhsT=med, rhs=exw[:, tl, :], start=first, stop=last
                    )
                    for h in range(HEADS):
                        nc.tensor.matmul(
                            pos[h // 2][:, (h % 2) * C : (h % 2 + 1) * C],
                            lhsT=ms[:, h],
                            rhs=gw[:, tl, h * C : (h + 1) * C],
                            start=first and (h % 2 == 0),
                            stop=last and (h % 2 == 1),
                        )

                t0 += twn

                # epilogue: yacc[:, c] = sum_h po[(4c+h)] * rec_h + bias
                den_s = sm.tile([128, 4], dt.float32, tag="den_s")
                nc.vector.tensor_scalar(den_s[:], den[:], 4.0, 1e-30, Alu.mult, Alu.add)
                rec = sm.tile([128, 4], dt.float32, tag="rec")
                nc.vector.reciprocal(rec[:], den_s[:])
                yacc = sm.tile([128, C], dt.float32, tag="yacc")
                for h in range(HEADS):
                    nc.vector.scalar_tensor_tensor(
                        yacc[:],
                        pos[h // 2][:, (h % 2) * C : (h % 2 + 1) * C],
                        rec[:, h : h + 1],
                        bia[:] if h == 0 else yacc[:],
                        Alu.mult,
                        Alu.add,
                    )
                nc.sync.dma_start(Y[w], yacc[:])

    _split_multiwaits(nc)
    return nc


def _host_prep(edge_index):
    """Static edge structure (depends only on edge_index, cached)."""
    ei = np.asarray(edge_index).astype(np.int64)
    loop = np.arange(N, dtype=np.int64)
    src = np.concatenate([ei[0], loop])
    dst = np.concatenate([ei[1], loop])

    core = dst // NPC
    dloc = dst - core * NPC
    win = dloc >> 7
    dstw = dloc & 127

    counts = np.zeros((NCORES, NWIN), np.int64)
    for j in range(NCORES):
        m = core == j
        cw = win[m]
        for w in range(NWIN):
            counts[j, w] = int((cw == w).sum())
    tw = [int(np.ceil(counts[:, w].max() / 128)) for w in range(NWIN)]
    twmax = max(tw)

    srcw = np.full((NCORES, NWIN, 128, twmax), PADROW, np.int32)
    mede = np.zeros((NCORES, NWIN, 128, twmax * 256), BF16)
    for jc in range(NCORES):
        m = core == jc
        sj, wj, dj = src[m], win[m], dstw[m]
        for w in range(NWIN):
            mw = wj == w
            cnt = int(mw.sum())
            s = np.asarray(sj[mw], np.int64)
            d = np.asarray(dj[mw], np.int64)
            jj, pp = np.divmod(np.arange(cnt), 128)
            srcw[jc, w, pp, jj] = s.astype(np.int32)
            oh = np.zeros((128, twmax, 256), np.float32)
            oh[pp, jj, d] = 1.0          # med: [e, d] one-hot
            oh[d, jj, 128 + pp] = 1.0    # mde: [d, e] one-hot
            mede[jc, w] = oh.reshape(128, twmax * 256).astype(BF16)

    adix = np.zeros((NCORES, NWIN, 128, 1), np.int32)
    iota = np.arange(128)
    for j in range(NCORES):
        for w in range(NWIN):
            rows = j * NPC + 128 * w + iota
            adix[j, w, :, 0] = np.minimum(rows, NPAD - 1)
    return tw, srcw, mede, adix


def _aug_weights(W, a_src, a_dst):
    W64 = np.asarray(W, np.float64)
    As = np.asarray(a_src, np.float64)
    Ad = np.asarray(a_dst, np.float64)
    Wh = W64.reshape(W64.shape[0], HEADS, C)
    wa_s = (Wh * As[None]).sum(-1)  # [K, HEADS]
    wa_d = (Wh * Ad[None]).sum(-1)
    waug = np.concatenate([W64, wa_s, wa_d], axis=1)  # [K, 1032]
    return waug.astype(BF16).reshape(2, 128, NAUG)


def _xt_pad(x):
    """x [N, 256] f32 -> XT bf16 [2, 128, NPAD] (zero-padded cols)."""
    xt = np.zeros((256, NPAD), np.float32)
    xt[:, :N] = np.asarray(x, np.float32).T
    return xt.astype(BF16).reshape(2, 128, NPAD)


def _layer_in_maps(x, W, a_src, a_dst, bias, srcw, mede, adix):
    xt = _xt_pad(x)
    waug = _aug_weights(W, a_src, a_dst)
    bias_b = np.broadcast_to(np.asarray(bias, np.float32)[None, :], (128, C)).copy()
    return [
        {
            "xt": xt,
            "waug": waug,
            "bias": bias_b,
            "src": srcw[j],
            "adix": adix[j],
            "mede": mede[j],
        }
        for j in range(NCORES)
    ]


def _run_layer(nc, in_maps):
    res = run_bass_kernel_spmd(nc, in_maps, core_ids=list(range(NCORES)))
    y = np.zeros((N, C), np.float32)
    for j in range(NCORES):
        yj = res.results[j]["y"]  # [NWIN, 128, C]
        y[j * NPC : j * NPC + 1024] = yj[:8].reshape(1024, C)
        y[j * NPC + 1024 : (j + 1) * NPC] = yj[8, :64]
    return y


def kernel(kpt_feature, edge_index, W1, a_src1, a_dst1, b1, W2, a_src2, a_dst2, b2):
    key = "k"
    if key not in _cache:
        tw, srcw, mede, adix = _host_prep(edge_index)
        nc = _build_layer_nc(tw)
        _cache[key] = (nc, tw, srcw, mede, adix)
    nc, tw, srcw, mede, adix = _cache[key]

    x1 = np.asarray(kpt_feature, np.float32).reshape(N, F)
    y1 = _run_layer(nc, _layer_in_maps(x1, W1, a_src1, a_dst1, b1, srcw, mede, adix))
    x2 = np.maximum(y1, 0.0)
    y2 = _run_layer(nc, _layer_in_maps(x2, W2, a_src2, a_dst2, b2, srcw, mede, adix))
    return y2.reshape(B, K, F).astype(np.float32)


# revision 23
# speedup vs baseline: 1.1284x; 1.1284x over previous
"""Two-layer GAT (KeypointGraph) on 8 Trainium2 NeuronCores.

Strategy (dst-sharded message passing):
 - Host: add self-loops, partition edges by destination node into 8 cores x
   1088 dst nodes, split each core's dsts into 9 windows of 128; pack each
   window's edge list into 128-edge tiles; build per-tile one-hot matrices
   med/mde fed as one batched bf16 input per window.
 - Device (one NEFF, run once per GAT layer, SPMD on 8 cores):
   Phase H: every core computes the full augmented feature matmul
     H = X @ [W | W@a_src | W@a_dst] -> table rows [h(1024)|asrc(4)] bf16 in
     DRAM plus adst(4) f32 table.  h columns are stored HEAD-MINOR (col =
     4*c+h) so the per-tile softmax scaling is one packed-bf16 DVE multiply.
   Phase E: per 128-edge tile, indirect-DMA row gather of [h|asrc] by src id,
     adst via one-hot matmul, logits e = leaky_relu(asrc+adst) (fused
     scalar_tensor_tensor), ex = exp(e) -> bf16, msg = ex*h via ONE broadcast
     multiply [128,1024], then one-hot matmuls accumulate the per-window
     denominator [128,4] and output [128,1024] in PSUM across tiles.
     Window epilogue: yacc = sum_h po[:, h::4]*rec_h + bias via fused
     scalar_tensor_tensor chain -> Y f32.
 - Host between layers: x2 = relu(y1) -> rerun same NEFF with layer-2 weights.
"""

import sys

sys.path.insert(0, "/opt/trn_rl_repo")

import numpy as np
import ml_dtypes

import concourse.bass as bass
import concourse.mybir as mybir
import concourse.tile as tile
from concourse.ap import AP
from concourse.bass import ts
from concourse.bass_utils import run_bass_kernel_spmd

BF16 = ml_dtypes.bfloat16

B, K, F = 512, 17, 256
N = B * K              # 8704
HEADS, C = 4, 256
HC = HEADS * C         # 1024
NAUG = HC + 8          # 1032
NCORES = 8
NPC = N // NCORES      # 1088 dst nodes per core
NWIN = 9               # 8 full 128-dst windows + 1 half window
NPAD = 8832            # node table rows (8704 real + pad row 8704 + align)
PADROW = N             # gather index for padding edges

_cache = {}


def _split_multiwaits(nc):
    """This image's walrus supports only ONE sync-wait command per
    instruction; hoist extra waits onto prepended same-engine NoOps."""
    for f in nc.m.functions:
        for blk in f.blocks:
            old = blk.instructions
            new = []
            changed = False
            for inst in old:
                si = inst.sync_info
                if si is not None and len(si.on_wait) > 1:
                    waits = list(si.on_wait)
                    for k, w in enumerate(waits[:-1]):
                        new.append(
                            mybir.InstNoOp(
                                name=f"{inst.name}_wsplit{k}",
                                engine=inst.engine,
                                sync_info=mybir.SyncInfo(on_wait=[w], on_update=[]),
                                bass_nofuse=True,
                            )
                        )
                    inst.sync_info = mybir.SyncInfo(
                        on_wait=[waits[-1]], on_update=list(si.on_update)
                    )
                    changed = True
                new.append(inst)
            if changed:
                blk.instructions = new


def _bcast_ap(t, offset_cols, reps, inner):
    """AP over tile t: [partitions, (0,reps), (1,inner)] at column offset."""
    full = t[:, :]
    dims = [list(full.ap[0]), [0, reps], [1, inner]]
    return AP(full.tensor, full.offset + offset_cols, dims)


def _strided_ap(t, offset_cols, stride, count):
    full = t[:, :]
    dims = [list(full.ap[0]), [stride, count]]
    return AP(full.tensor, full.offset + offset_cols, dims)


def _build_layer_nc(tw):
    """One GAT layer, SPMD over 8 cores. tw: tiles per window (len NWIN)."""
    nc = bass.Bass(num_devices=NCORES)
    dt = mybir.dt
    Alu = mybir.AluOpType
    Act = mybir.ActivationFunctionType
    twmax = max(tw)

    XT = nc.dram_tensor("xt", [2, 128, NPAD], dt.bfloat16, kind="ExternalInput")
    WAUG = nc.dram_tensor("waug", [2, 128, NAUG], dt.bfloat16, kind="ExternalInput")
    BIAS = nc.dram_tensor("bias", [128, C], dt.float32, kind="ExternalInput")
    SRC = nc.dram_tensor("src", [NWIN, 128, twmax], dt.int32, kind="ExternalInput")
    ADIX = nc.dram_tensor("adix", [NWIN, 128, 1], dt.int32, kind="ExternalInput")
    MEDE = nc.dram_tensor("mede", [NWIN, 128, twmax * 256], dt.bfloat16, kind="ExternalInput")
    Y = nc.dram_tensor("y", [NWIN, 128, C], dt.float32, kind="ExternalOutput")

    TROWB = HC + 4  # bf16 elems: 1024 h | 4 asrc
    HTAB = nc.dram_tensor("htab", [NPAD, TROWB], dt.bfloat16)
    ADSTT = nc.dram_tensor("adstt", [NPAD, 4], dt.float32)

    with tile.TileContext(nc) as tc:
        with (
            tc.tile_pool(name="per", bufs=1) as per,
            tc.tile_pool(name="hsb", bufs=3) as hpool,
            tc.tile_pool(name="ed", bufs=8) as ed,
            tc.tile_pool(name="mp", bufs=2) as mp,
            tc.tile_pool(name="sm", bufs=8) as sm,
            tc.tile_pool(name="msp", bufs=6) as msp,
            tc.tile_pool(name="pph", bufs=4, space="PSUM") as pph,
            tc.tile_pool(name="pp1", bufs=1, space="PSUM") as pp1,
            tc.tile_pool(name="ppd", bufs=1, space="PSUM") as ppd,
            tc.tile_pool(name="pp2", bufs=1, space="PSUM") as pp2,
        ):
            xts = []
            for k in range(2):
                x = per.tile([128, NPAD], dt.bfloat16, tag=f"xt{k}")
                nc.sync.dma_start(x[:], XT[k])
                xts.append(x)
            wgs = []
            for k in range(2):
                w = per.tile([128, NAUG], dt.bfloat16, tag=f"wg{k}")
                nc.sync.dma_start(w[:], WAUG[k])
                wgs.append(w)
            bia = per.tile([128, C], dt.float32, tag="bias")
            nc.sync.dma_start(bia[:], BIAS[:])

            # ---- Phase H: augmented feature matmul into DRAM tables ----
            # GRP blocks staged per hsb tile, written with one (transposed-AP)
            # DMA each for htab and adstt.
            GRP = 4
            HT3 = HTAB[:, :].tensor
            for g0 in range(0, NPAD // 128, GRP):
                gn = min(GRP, NPAD // 128 - g0)
                hsb = hpool.tile([128, GRP, TROWB], dt.bfloat16, tag="hsb")
                asb = hpool.tile([128, GRP, 4], dt.float32, tag="asb")
                for b in range(gn):
                    nb = g0 + b
                    for c0, cn in ((0, 512), (512, 512), (1024, 8)):
                        ps = pph.tile([128, cn], dt.float32, tag="hps")
                        for k in range(2):
                            nc.tensor.matmul(
                                ps[:],
                                lhsT=xts[k][:, ts(nb, 128)],
                                rhs=wgs[k][:, c0 : c0 + cn],
                                start=(k == 0),
                                stop=(k == 1),
                            )
                        if cn == 512:
                            if (c0 == 0) == (nb % 2 == 0):
                                nc.scalar.copy(hsb[:, b, c0 : c0 + 512], ps[:])
                            else:
                                nc.vector.tensor_copy(hsb[:, b, c0 : c0 + 512], ps[:])
                        else:
                            nc.scalar.copy(hsb[:, b, 1024:1028], ps[:, 0:4])
                            nc.scalar.copy(asb[:, b, :], ps[:, 4:8])
                # htab rows g0*128 .. (g0+gn)*128: [part, block, col] AP
                ho = AP(
                    HTAB[:, :].tensor,
                    g0 * 128 * TROWB,
                    [[TROWB, 128], [128 * TROWB, gn], [1, TROWB]],
                )
                nc.sync.dma_start(ho, hsb[:, 0:gn, :])
                ao = AP(
                    ADSTT[:, :].tensor,
                    g0 * 128 * 4,
                    [[4, 128], [128 * 4, gn], [1, 4]],
                )
                nc.sync.dma_start(ao, asb[:, 0:gn, :])

            # ---- Phase E: per-window edge aggregation ----
            t0 = 0
            for w in range(NWIN):
                twn = tw[w]
                aidx = sm.tile([128, 1], dt.int32, tag="aidx")
                nc.sync.dma_start(aidx[:], ADIX[w])
                adw = sm.tile([128, 4], dt.float32, tag="adw")
                nc.gpsimd.indirect_dma_start(
                    out=adw[:],
                    out_offset=None,
                    in_=ADSTT[:, :],
                    in_offset=bass.IndirectOffsetOnAxis(ap=aidx[:, :1], axis=0),
                )
                adwb = sm.tile([128, 4], dt.bfloat16, tag="adwb")
                nc.vector.tensor_copy(adwb[:], adw[:])

                po0 = pp1.tile([128, 512], dt.float32, tag="po0")
                po1 = pp1.tile([128, 512], dt.float32, tag="po1")
                pos = (po0, po1)
                den = ppd.tile([128, 4], dt.float32, tag="den")

                sidxw = sm.tile([128, twmax], dt.int32, tag="sidxw")
                nc.sync.dma_start(sidxw[:], SRC[w])
                mt = mp.tile([128, twmax * 256], dt.bfloat16, tag="mt")
                nc.sync.dma_start(mt[:, : twn * 256], MEDE[w][:, : twn * 256])

                # gather all tiles' rows into one window tile (one single-index
                # indirect DMA per 128-edge tile, strided output slices)
                gw = mp.tile([128, twmax, TROWB], dt.bfloat16, tag="gw")
                for tl in range(twn):
                    nc.gpsimd.indirect_dma_start(
                        out=gw[:, tl, :],
                        out_offset=None,
                        in_=HTAB[:, :],
                        in_offset=bass.IndirectOffsetOnAxis(
                            ap=sidxw[:, tl : tl + 1], axis=0
                        ),
                    )

                # batched per-window logits: psa into col slices of one bank
                psa = pp2.tile([128, 4 * twmax], dt.float32, tag="psa")
                for tl in range(twn):
                    nc.tensor.matmul(
                        psa[:, 4 * tl : 4 * tl + 4],
                        lhsT=mt[:, tl * 256 + 128 : tl * 256 + 256],
                        rhs=adwb[:],
                        start=True,
                        stop=True,
                    )
                ef = sm.tile([128, twmax, 4], dt.float32, tag="ef")
                nc.vector.tensor_add(
                    ef[:, 0:twn],
                    gw[:, 0:twn, HC : HC + 4],
                    AP(psa.tensor, psa.offset, [list(psa.ap[0]), [4, twn], [1, 4]]),
                )
                nc.vector.scalar_tensor_tensor(
                    ef[:, 0:twn], ef[:, 0:twn], 0.2, ef[:, 0:twn], Alu.mult, Alu.max
                )
                exw = sm.tile([128, twmax, 4], dt.bfloat16, tag="exw")
                nc.scalar.activation(exw[:, 0:twn], ef[:, 0:twn], Act.Exp)

                for tl in range(twn):
                    first = tl == 0
                    last = tl == twn - 1
                    med = mt[:, tl * 256 : tl * 256 + 128]
                    # scale the ONE-HOT (not the messages): ms[,h,d] =
                    # med[,d] * ex[,h] in one fused mul (med repeated 4x via
                    # stride-0 outer dim), so the gathered rows feed matmuls
                    # unscaled (enables fp8 rhs).
                    ms = msp.tile([128, HEADS, 128], dt.bfloat16, tag="ms")
                    mtf = mt[:, :]
                    exf = exw[:, :]
                    in1 = AP(mtf.tensor, mtf.offset + tl * 256,
                             [list(mtf.ap[0]), [0, HEADS], [1, 128]])
                    in2 = AP(exf.tensor, exf.offset + 4 * tl,
                             [list(exf.ap[0]), [1, HEADS], [0, 128]])
                    nc.vector.tensor_mul(ms[:], in1, in2)
                    nc.tensor.matmul(
                        den[:], lhsT=med, rhs=exw[:, tl, :], start=first, stop=last
                    )
                    for h in range(HEADS):
                        nc.tensor.matmul(
                            pos[h // 2][:, (h % 2) * C : (h % 2 + 1) * C],
                            lhsT=ms[:, h],
                            rhs=gw[:, tl, h * C : (h + 1) * C],
                            start=first and (h % 2 == 0),
                            stop=last and (h % 2 == 1),
                        )

                t0 += twn

                # epilogue: yacc[:, c] = sum_h po[(4c+h)] * rec_h + bias
                den_s = sm.tile([128, 4], dt.float32, tag="den_s")
                nc.vector.tensor_scalar(den_s[:], den[:], 4.0, 1e-30, Alu.mult, Alu.add)
                rec = sm.tile([128, 4], dt.float32, tag="rec")
                nc.vector.reciprocal(rec[:], den_s[:])
                yacc = sm.tile([128, C], dt.float32, tag="yacc")
                for h in range(HEADS):
                    nc.vector.scalar_tensor_tensor(
                        yacc[:],
                        pos[h // 2][:, (h % 2) * C : (h % 2 + 1) * C],
                        rec[:, h : h + 1],
                        bia[:] if h == 0 else yacc[:],
                        Alu.mult,
                        Alu.add,
                    )
                nc.sync.dma_start(Y[w], yacc[:])

    _split_multiwaits(nc)
    return nc


def _host_prep(edge_index):
    """Static edge structure (depends only on edge_index, cached)."""
    ei = np.asarray(edge_index).astype(np.int64)
    loop = np.arange(N, dtype=np.int64)
    src = np.concatenate([ei[0], loop])
    dst = np.concatenate([ei[1], loop])

    core = dst // NPC
    dloc = dst - core * NPC
    win = dloc >> 7
    dstw = dloc & 127

    counts = np.zeros((NCORES, NWIN), np.int64)
    for j in range(NCORES):
        m = core == j
        cw = win[m]
        for w in range(NWIN):
            counts[j, w] = int((cw == w).sum())
    tw = [int(np.ceil(counts[:, w].max() / 128)) for w in range(NWIN)]
    twmax = max(tw)

    srcw = np.full((NCORES, NWIN, 128, twmax), PADROW, np.int32)
    mede = np.zeros((NCORES, NWIN, 128, twmax * 256), BF16)
    for jc in range(NCORES):
        m = core == jc
        sj, wj, dj = src[m], win[m], dstw[m]
        for w in range(NWIN):
            mw = wj == w
            cnt = int(mw.sum())
            s = np.asarray(sj[mw], np.int64)
            d = np.asarray(dj[mw], np.int64)
            jj, pp = np.divmod(np.arange(cnt), 128)
            srcw[jc, w, pp, jj] = s.astype(np.int32)
            oh = np.zeros((128, twmax, 256), np.float32)
            oh[pp, jj, d] = 1.0          # med: [e, d] one-hot
            oh[d, jj, 128 + pp] = 1.0    # mde: [d, e] one-hot
            mede[jc, w] = oh.reshape(128, twmax * 256).astype(BF16)

    adix = np.zeros((NCORES, NWIN, 128, 1), np.int32)
    iota = np.arange(128)
    for j in range(NCORES):
        for w in range(NWIN):
            rows = j * NPC + 128 * w + iota
            adix[j, w, :, 0] = np.minimum(rows, NPAD - 1)
    return tw, srcw, mede, adix


def _aug_weights(W, a_src, a_dst):
    W64 = np.asarray(W, np.float64)
    As = np.asarray(a_src, np.float64)
    Ad = np.asarray(a_dst, np.float64)
    Wh = W64.reshape(W64.shape[0], HEADS, C)
    wa_s = (Wh * As[None]).sum(-1)  # [K, HEADS]
    wa_d = (Wh * Ad[None]).sum(-1)
    waug = np.concatenate([W64, wa_s, wa_d], axis=1)  # [K, 1032]
    return waug.astype(BF16).reshape(2, 128, NAUG)


def _xt_pad(x):
    """x [N, 256] f32 -> XT bf16 [2, 128, NPAD] (zero-padded cols)."""
    xt = np.zeros((256, NPAD), np.float32)
    xt[:, :N] = np.asarray(x, np.float32).T
    return xt.astype(BF16).reshape(2, 128, NPAD)


def _layer_in_maps(x, W, a_src, a_dst, bias, srcw, mede, adix):
    xt = _xt_pad(x)
    waug = _aug_weights(W, a_src, a_dst)
    bias_b = np.broadcast_to(np.asarray(bias, np.float32)[None, :], (128, C)).copy()
    return [
        {
            "xt": xt,
            "waug": waug,
            "bias": bias_b,
            "src": srcw[j],
            "adix": adix[j],
            "mede": mede[j],
        }
        for j in range(NCORES)
    ]


def _run_layer(nc, in_maps):
    res = run_bass_kernel_spmd(nc, in_maps, core_ids=list(range(NCORES)))
    y = np.zeros((N, C), np.float32)
    for j in range(NCORES):
        yj = res.results[j]["y"]  # [NWIN, 128, C]
        y[j * NPC : j * NPC + 1024] = yj[:8].reshape(1024, C)
        y[j * NPC + 1024 : (j + 1) * NPC] = yj[8, :64]
    return y


def kernel(kpt_feature, edge_index, W1, a_src1, a_dst1, b1, W2, a_src2, a_dst2, b2):
    key = "k"
    if key not in _cache:
        tw, srcw, mede, adix = _host_prep(edge_index)
        nc = _build_layer_nc(tw)
        _cache[key] = (nc, tw, srcw, mede, adix)
    nc, tw, srcw, mede, adix = _cache[key]

    x1 = np.asarray(kpt_feature, np.float32).reshape(N, F)
    y1 = _run_layer(nc, _layer_in_maps(x1, W1, a_src1, a_dst1, b1, srcw, mede, adix))
    x2 = np.maximum(y1, 0.0)
    y2 = _run_layer(nc, _layer_in_maps(x2, W2, a_src2, a_dst2, b2, srcw, mede, adix))
    return y2.reshape(B, K, F).astype(np.float32)


# revision 24
# speedup vs baseline: 1.1921x; 1.0564x over previous
"""Two-layer GAT (KeypointGraph) on 8 Trainium2 NeuronCores.

Strategy (dst-sharded message passing):
 - Host: add self-loops, partition edges by destination node into 8 cores x
   1088 dst nodes, split each core's dsts into 9 windows of 128; pack each
   window's edge list into 128-edge tiles; build per-tile one-hot matrices
   med/mde fed as one batched bf16 input per window.
 - Device (one NEFF, run once per GAT layer, SPMD on 8 cores):
   Phase H: every core computes the full augmented feature matmul
     H = X @ [W | W@a_src | W@a_dst] -> table rows [h(1024)|asrc(4)] bf16 in
     DRAM plus adst(4) f32 table.  h columns are stored HEAD-MINOR (col =
     4*c+h) so the per-tile softmax scaling is one packed-bf16 DVE multiply.
   Phase E: per 128-edge tile, indirect-DMA row gather of [h|asrc] by src id,
     adst via one-hot matmul, logits e = leaky_relu(asrc+adst) (fused
     scalar_tensor_tensor), ex = exp(e) -> bf16, msg = ex*h via ONE broadcast
     multiply [128,1024], then one-hot matmuls accumulate the per-window
     denominator [128,4] and output [128,1024] in PSUM across tiles.
     Window epilogue: yacc = sum_h po[:, h::4]*rec_h + bias via fused
     scalar_tensor_tensor chain -> Y f32.
 - Host between layers: x2 = relu(y1) -> rerun same NEFF with layer-2 weights.
"""

import sys

sys.path.insert(0, "/opt/trn_rl_repo")

import numpy as np
import ml_dtypes

import concourse.bass as bass
import concourse.mybir as mybir
import concourse.tile as tile
from concourse.ap import AP
from concourse.bass import ts
from concourse.bass_utils import run_bass_kernel_spmd

BF16 = ml_dtypes.bfloat16

B, K, F = 512, 17, 256
N = B * K              # 8704
HEADS, C = 4, 256
HC = HEADS * C         # 1024
NAUG = HC + 8          # 1032
NCORES = 8
NPC = N // NCORES      # 1088 dst nodes per core
NWIN = 9               # 8 full 128-dst windows + 1 half window
NPAD = 8832            # node table rows (8704 real + pad row 8704 + align)
PADROW = N             # gather index for padding edges

_cache = {}


def _split_multiwaits(nc):
    """This image's walrus supports only ONE sync-wait command per
    instruction; hoist extra waits onto prepended same-engine NoOps."""
    for f in nc.m.functions:
        for blk in f.blocks:
            old = blk.instructions
            new = []
            changed = False
            for inst in old:
                si = inst.sync_info
                if si is not None and len(si.on_wait) > 1:
                    waits = list(si.on_wait)
                    for k, w in enumerate(waits[:-1]):
                        new.append(
                            mybir.InstNoOp(
                                name=f"{inst.name}_wsplit{k}",
                                engine=inst.engine,
                                sync_info=mybir.SyncInfo(on_wait=[w], on_update=[]),
                                bass_nofuse=True,
                            )
                        )
                    inst.sync_info = mybir.SyncInfo(
                        on_wait=[waits[-1]], on_update=list(si.on_update)
                    )
                    changed = True
                new.append(inst)
            if changed:
                blk.instructions = new


def _bcast_ap(t, offset_cols, reps, inner):
    """AP over tile t: [partitions, (0,reps), (1,inner)] at column offset."""
    full = t[:, :]
    dims = [list(full.ap[0]), [0, reps], [1, inner]]
    return AP(full.tensor, full.offset + offset_cols, dims)


def _strided_ap(t, offset_cols, stride, count):
    full = t[:, :]
    dims = [list(full.ap[0]), [stride, count]]
    return AP(full.tensor, full.offset + offset_cols, dims)


def _build_layer_nc(tw):
    """One GAT layer, SPMD over 8 cores. tw: tiles per window (len NWIN)."""
    nc = bass.Bass(num_devices=NCORES)
    dt = mybir.dt
    Alu = mybir.AluOpType
    Act = mybir.ActivationFunctionType
    twmax = max(tw)

    XT = nc.dram_tensor("xt", [2, 128, NPAD], dt.bfloat16, kind="ExternalInput")
    WAUG = nc.dram_tensor("waug", [2, 128, NAUG], dt.bfloat16, kind="ExternalInput")
    BIAS = nc.dram_tensor("bias", [128, C], dt.float32, kind="ExternalInput")
    SRC = nc.dram_tensor("src", [NWIN, 128, twmax], dt.int32, kind="ExternalInput")
    ADIX = nc.dram_tensor("adix", [NWIN, 128, 1], dt.int32, kind="ExternalInput")
    MEDE = nc.dram_tensor("mede", [NWIN, 128, twmax * 256], dt.bfloat16, kind="ExternalInput")
    Y = nc.dram_tensor("y", [NWIN, 128, C], dt.float32, kind="ExternalOutput")

    TROWB = HC + 8  # bf16 elems: 1024 h | 4 asrc | 4 adst
    HTAB = nc.dram_tensor("htab", [NPAD, TROWB], dt.bfloat16)

    with tile.TileContext(nc) as tc:
        with (
            tc.tile_pool(name="per", bufs=1) as per,
            tc.tile_pool(name="hsb", bufs=3) as hpool,
            tc.tile_pool(name="ed", bufs=8) as ed,
            tc.tile_pool(name="mp", bufs=2) as mp,
            tc.tile_pool(name="sm", bufs=8) as sm,
            tc.tile_pool(name="msp", bufs=6) as msp,
            tc.tile_pool(name="pph", bufs=4, space="PSUM") as pph,
            tc.tile_pool(name="pp1", bufs=1, space="PSUM") as pp1,
            tc.tile_pool(name="ppd", bufs=1, space="PSUM") as ppd,
            tc.tile_pool(name="pp2", bufs=1, space="PSUM") as pp2,
        ):
            xts = []
            for k in range(2):
                x = per.tile([128, NPAD], dt.bfloat16, tag=f"xt{k}")
                nc.sync.dma_start(x[:], XT[k])
                xts.append(x)
            wgs = []
            for k in range(2):
                w = per.tile([128, NAUG], dt.bfloat16, tag=f"wg{k}")
                nc.sync.dma_start(w[:], WAUG[k])
                wgs.append(w)
            bia = per.tile([128, C], dt.float32, tag="bias")
            nc.sync.dma_start(bia[:], BIAS[:])

            # ---- Phase H: augmented feature matmul into DRAM tables ----
            # GRP blocks staged per hsb tile, written with one (transposed-AP)
            # DMA each for htab and adstt.
            GRP = 4
            HT3 = HTAB[:, :].tensor
            for g0 in range(0, NPAD // 128, GRP):
                gn = min(GRP, NPAD // 128 - g0)
                hsb = hpool.tile([128, GRP, TROWB], dt.bfloat16, tag="hsb")
                for b in range(gn):
                    nb = g0 + b
                    for c0, cn in ((0, 512), (512, 512), (1024, 8)):
                        ps = pph.tile([128, cn], dt.float32, tag="hps")
                        for k in range(2):
                            nc.tensor.matmul(
                                ps[:],
                                lhsT=xts[k][:, ts(nb, 128)],
                                rhs=wgs[k][:, c0 : c0 + cn],
                                start=(k == 0),
                                stop=(k == 1),
                            )
                        if cn == 512:
                            if (c0 == 0) == (nb % 2 == 0):
                                nc.scalar.copy(hsb[:, b, c0 : c0 + 512], ps[:])
                            else:
                                nc.vector.tensor_copy(hsb[:, b, c0 : c0 + 512], ps[:])
                        elif nb % 2 == 0:
                            nc.vector.tensor_copy(hsb[:, b, 1024:1032], ps[:, 0:8])
                        else:
                            nc.scalar.copy(hsb[:, b, 1024:1032], ps[:, 0:8])
                # htab rows g0*128 .. (g0+gn)*128: [part, block, col] AP
                ho = AP(
                    HTAB[:, :].tensor,
                    g0 * 128 * TROWB,
                    [[TROWB, 128], [128 * TROWB, gn], [1, TROWB]],
                )
                nc.sync.dma_start(ho, hsb[:, 0:gn, :])

            # ---- Phase E: per-window edge aggregation ----
            t0 = 0
            for w in range(NWIN):
                twn = tw[w]
                aidx = sm.tile([128, 1], dt.int32, tag="aidx")
                nc.sync.dma_start(aidx[:], ADIX[w])
                adwb = sm.tile([128, 4], dt.bfloat16, tag="adwb")
                nc.gpsimd.indirect_dma_start(
                    out=adwb[:],
                    out_offset=None,
                    in_=HTAB[:, :],
                    in_offset=bass.IndirectOffsetOnAxis(ap=aidx[:, :1], axis=0),
                    element_offset=HC + 4,
                )

                po0 = pp1.tile([128, 512], dt.float32, tag="po0")
                po1 = pp1.tile([128, 512], dt.float32, tag="po1")
                pos = (po0, po1)
                den = ppd.tile([128, 4], dt.float32, tag="den")

                sidxw = sm.tile([128, twmax], dt.int32, tag="sidxw")
                nc.sync.dma_start(sidxw[:], SRC[w])
                mt = mp.tile([128, twmax * 256], dt.bfloat16, tag="mt")
                nc.sync.dma_start(mt[:, : twn * 256], MEDE[w][:, : twn * 256])

                # gather all tiles' rows into one window tile (one single-index
                # indirect DMA per 128-edge tile, strided output slices)
                gw = mp.tile([128, twmax, TROWB], dt.bfloat16, tag="gw")
                for tl in range(twn):
                    nc.gpsimd.indirect_dma_start(
                        out=gw[:, tl, :],
                        out_offset=None,
                        in_=HTAB[:, :],
                        in_offset=bass.IndirectOffsetOnAxis(
                            ap=sidxw[:, tl : tl + 1], axis=0
                        ),
                    )

                # batched per-window logits: psa into col slices of one bank
                psa = pp2.tile([128, 4 * twmax], dt.float32, tag="psa")
                for tl in range(twn):
                    nc.tensor.matmul(
                        psa[:, 4 * tl : 4 * tl + 4],
                        lhsT=mt[:, tl * 256 + 128 : tl * 256 + 256],
                        rhs=adwb[:],
                        start=True,
                        stop=True,
                    )
                ef = sm.tile([128, twmax, 4], dt.float32, tag="ef")
                nc.vector.tensor_add(
                    ef[:, 0:twn],
                    gw[:, 0:twn, HC : HC + 4],
                    AP(psa.tensor, psa.offset, [list(psa.ap[0]), [4, twn], [1, 4]]),
                )
                nc.vector.scalar_tensor_tensor(
                    ef[:, 0:twn], ef[:, 0:twn], 0.2, ef[:, 0:twn], Alu.mult, Alu.max
                )
                exw = sm.tile([128, twmax, 4], dt.bfloat16, tag="exw")
                nc.scalar.activation(exw[:, 0:twn], ef[:, 0:twn], Act.Exp)

                for tl in range(twn):
                    first = tl == 0
                    last = tl == twn - 1
                    med = mt[:, tl * 256 : tl * 256 + 128]
                    # scale the ONE-HOT (not the messages): ms[,h,d] =
                    # med[,d] * ex[,h] in one fused mul (med repeated 4x via
                    # stride-0 outer dim), so the gathered rows feed matmuls
                    # unscaled (enables fp8 rhs).
                    ms = msp.tile([128, HEADS, 128], dt.bfloat16, tag="ms")
                    mtf = mt[:, :]
                    exf = exw[:, :]
                    in1 = AP(mtf.tensor, mtf.offset + tl * 256,
                             [list(mtf.ap[0]), [0, HEADS], [1, 128]])
                    in2 = AP(exf.tensor, exf.offset + 4 * tl,
                             [list(exf.ap[0]), [1, HEADS], [0, 128]])
                    nc.vector.tensor_mul(ms[:], in1, in2)
                    nc.tensor.matmul(
                        den[:], lhsT=med, rhs=exw[:, tl, :], start=first, stop=last
                    )
                    for h in range(HEADS):
                        nc.tensor.matmul(
                            pos[h // 2][:, (h % 2) * C : (h % 2 + 1) * C],
                            lhsT=ms[:, h],
                            rhs=gw[:, tl, h * C : (h + 1) * C],
                            start=first and (h % 2 == 0),
                            stop=last and (h % 2 == 1),
                        )

                t0 += twn

                # epilogue: yacc[:, c] = sum_h po[(4c+h)] * rec_h + bias
                den_s = sm.tile([128, 4], dt.float32, tag="den_s")
                nc.vector.tensor_scalar(den_s[:], den[:], 4.0, 1e-30, Alu.mult, Alu.add)
                rec = sm.tile([128, 4], dt.float32, tag="rec")
                nc.vector.reciprocal(rec[:], den_s[:])
                yacc = sm.tile([128, C], dt.float32, tag="yacc")
                for h in range(HEADS):
                    nc.vector.scalar_tensor_tensor(
                        yacc[:],
                        pos[h // 2][:, (h % 2) * C : (h % 2 + 1) * C],
                        rec[:, h : h + 1],
                        bia[:] if h == 0 else yacc[:],
                        Alu.mult,
                        Alu.add,
                    )
                nc.sync.dma_start(Y[w], yacc[:])

    _split_multiwaits(nc)
    return nc


def _host_prep(edge_index):
    """Static edge structure (depends only on edge_index, cached)."""
    ei = np.asarray(edge_index).astype(np.int64)
    loop = np.arange(N, dtype=np.int64)
    src = np.concatenate([ei[0], loop])
    dst = np.concatenate([ei[1], loop])

    core = dst // NPC
    dloc = dst - core * NPC
    win = dloc >> 7
    dstw = dloc & 127

    counts = np.zeros((NCORES, NWIN), np.int64)
    for j in range(NCORES):
        m = core == j
        cw = win[m]
        for w in range(NWIN):
            counts[j, w] = int((cw == w).sum())
    tw = [int(np.ceil(counts[:, w].max() / 128)) for w in range(NWIN)]
    twmax = max(tw)

    srcw = np.full((NCORES, NWIN, 128, twmax), PADROW, np.int32)
    mede = np.zeros((NCORES, NWIN, 128, twmax * 256), BF16)
    for jc in range(NCORES):
        m = core == jc
        sj, wj, dj = src[m], win[m], dstw[m]
        for w in range(NWIN):
            mw = wj == w
            cnt = int(mw.sum())
            s = np.asarray(sj[mw], np.int64)
            d = np.asarray(dj[mw], np.int64)
            jj, pp = np.divmod(np.arange(cnt), 128)
            srcw[jc, w, pp, jj] = s.astype(np.int32)
            oh = np.zeros((128, twmax, 256), np.float32)
            oh[pp, jj, d] = 1.0          # med: [e, d] one-hot
            oh[d, jj, 128 + pp] = 1.0    # mde: [d, e] one-hot
            mede[jc, w] = oh.reshape(128, twmax * 256).astype(BF16)

    adix = np.zeros((NCORES, NWIN, 128, 1), np.int32)
    iota = np.arange(128)
    for j in range(NCORES):
        for w in range(NWIN):
            rows = j * NPC + 128 * w + iota
            adix[j, w, :, 0] = np.minimum(rows, NPAD - 1)
    return tw, srcw, mede, adix


def _aug_weights(W, a_src, a_dst):
    W64 = np.asarray(W, np.float64)
    As = np.asarray(a_src, np.float64)
    Ad = np.asarray(a_dst, np.float64)
    Wh = W64.reshape(W64.shape[0], HEADS, C)
    wa_s = (Wh * As[None]).sum(-1)  # [K, HEADS]
    wa_d = (Wh * Ad[None]).sum(-1)
    waug = np.concatenate([W64, wa_s, wa_d], axis=1)  # [K, 1032]
    return waug.astype(BF16).reshape(2, 128, NAUG)


def _xt_pad(x):
    """x [N, 256] f32 -> XT bf16 [2, 128, NPAD] (zero-padded cols)."""
    xt = np.zeros((256, NPAD), np.float32)
    xt[:, :N] = np.asarray(x, np.float32).T
    return xt.astype(BF16).reshape(2, 128, NPAD)


def _layer_in_maps(x, W, a_src, a_dst, bias, srcw, mede, adix):
    xt = _xt_pad(x)
    waug = _aug_weights(W, a_src, a_dst)
    bias_b = np.broadcast_to(np.asarray(bias, np.float32)[None, :], (128, C)).copy()
    return [
        {
            "xt": xt,
            "waug": waug,
            "bias": bias_b,
            "src": srcw[j],
            "adix": adix[j],
            "mede": mede[j],
        }
        for j in range(NCORES)
    ]


def _run_layer(nc, in_maps):
    res = run_bass_kernel_spmd(nc, in_maps, core_ids=list(range(NCORES)))
    y = np.zeros((N, C), np.float32)
    for j in range(NCORES):
        yj = res.results[j]["y"]  # [NWIN, 128, C]
        y[j * NPC : j * NPC + 1024] = yj[:8].reshape(1024, C)
        y[j * NPC + 1024 : (j + 1) * NPC] = yj[8, :64]
    return y


def kernel(kpt_feature, edge_index, W1, a_src1, a_dst1, b1, W2, a_src2, a_dst2, b2):
    key = "k"
    if key not in _cache:
        tw, srcw, mede, adix = _host_prep(edge_index)
        nc = _build_layer_nc(tw)
        _cache[key] = (nc, tw, srcw, mede, adix)
    nc, tw, srcw, mede, adix = _cache[key]

    x1 = np.asarray(kpt_feature, np.float32).reshape(N, F)
    y1 = _run_layer(nc, _layer_in_maps(x1, W1, a_src1, a_dst1, b1, srcw, mede, adix))
    x2 = np.maximum(y1, 0.0)
    y2 = _run_layer(nc, _layer_in_maps(x2, W2, a_src2, a_dst2, b2, srcw, mede, adix))
    return y2.reshape(B, K, F).astype(np.float32)
